# revision 8
# baseline (speedup 1.0000x reference)
"""Causal MHSA Trainium2 kernel (8 NeuronCores) — v6.

Sharding: core c = 4*b + g handles batch b and head-group g (4 of 16
heads); host sums the 4 head-group partial projections per batch.

v6 (vs v3, 162.7us -> 124.0us):
- QKV projections run as fp8 DoubleRow matmuls (2 kt-tiles per
  instruction at 0.5 cycles/col) with residual compensation:
  W^T x ~= W8^T x8 + W8^T dx8 + dW8^T x8, main terms e4m3, residuals
  e5m2 (e4m3 residuals flush to subnormals and break the correction).
  All quantization happens on the host; 3 compensated terms cost 0.75x
  a bf16 pass and hold end-to-end rel err at 4.5e-3.
- Everything else streams bf16: host-prepared partition-major SBUF
  images for all inputs, bf16 q_T/k_T/ctx_T (diagonal score matmuls at
  exact width), bf16 output partials (host upcasts and sums).
- In-block scheduling: filler units pop BEFORE each dependency-carrying
  ctx matmul (PE executes in order, so a stalled ctx would block
  independent work queued behind it), with adaptive pop counts that
  drain the filler list evenly across the block.
- Tail: the last block's norm muls are column-split so st12/13 out-proj
  starts after the first half; broadcast matmuls take the freed "s"
  PSUM slots; post-loop drain copies go to ACT; staging copies split
  ACT/DVE and out-DMAs alternate HWDGE/SWDGE queues.
"""

import json

import ml_dtypes
import numpy as np

import concourse.bass as bass
import concourse.mybir as mybir
import concourse.tile as tile
from concourse.bass_utils import run_bass_kernel_spmd

# ---------------------------------------------------------------------------
# Workaround: this container's walrus rejects instructions carrying more
# than one semaphore wait ("Too many sync wait commands", e.g. on the
# TileContext final drain). Split every multi-wait instruction into
# single-wait NoOps on the same engine placed immediately before it.
# ---------------------------------------------------------------------------


def _split_multiwait_bir(bir_bytes: bytes) -> bytes:
    bir = json.loads(bir_bytes)
    ctr = 0
    for fn in bir.get("functions", []):
        for bb in fn.get("blocks", []):
            out = []
            for inst in bb.get("instructions", []):
                si = inst.get("sync_info")
                waits = (si or {}).get("on_wait") or []
                if len(waits) > 1 and "engine" in inst:
                    for w in waits:
                        ctr += 1
                        out.append(
                            {
                                "debug": inst.get("debug", 0),
                                "engine": inst["engine"],
                                "ins": [],
                                "outs": [],
                                "name": f"{inst['name']}-sw{ctr}",
                                "opcode": "NoOp",
                                "sync_info": {"on_update": [], "on_wait": [w]},
                            }
                        )
                    si["on_wait"] = []
                out.append(inst)
            bb["instructions"] = out
    return json.dumps(bir).encode()


class _BassSplitWaits(bass.Bass):
    def to_json_bytes(self) -> bytes:
        return _split_multiwait_bir(super().to_json_bytes())


# ---------------------------------------------------------------------------
B = 2
S = 2048
D = 1024
HD = 64
N_CORES = 8
NHL = 4  # heads per core
E = NHL * HD  # 256
DT = D // 128  # 8
ST = S // 128  # 16
QBS = 512
NQB = S // QBS  # 4
F32 = mybir.dt.float32
F32R = mybir.dt.float32r
BF16 = mybir.dt.bfloat16
E4M3 = mybir.dt.float8e4
E5M2 = mybir.dt.float8e5
DR = mybir.MatmulPerfMode.DoubleRow
SCALE = 1.0 / np.sqrt(HD)


def build_nc() -> bass.Bass:
    nc = _BassSplitWaits()

    # host-prepared SBUF images (partition-major). QKV runs as fp8
    # DoubleRow with residual compensation: W^T x ~= W8^T x8 + W8^T dx8 +
    # dW8^T x8, where *8 are e4m3 and d* are e5m2 residuals (r = full - *8).
    # Layouts pair kt tiles for DoubleRow: x images are [p, j, t, s]
    # (kt = 2j + t), w images [p, j, t, e].
    NJ = DT // 2  # 4 kt-pairs
    x8_img = nc.dram_tensor("x8_img", [128, DT * S], E4M3, kind="ExternalInput")
    dx8_img = nc.dram_tensor("dx8_img", [128, DT * S], E5M2, kind="ExternalInput")
    w8_imgs = {
        w: nc.dram_tensor(f"{w}8_img", [128, DT * E], E4M3, kind="ExternalInput")
        for w in ("wq", "wk", "wv")
    }
    dw8_imgs = {
        w: nc.dram_tensor(f"d{w}8_img", [128, DT * E], E5M2, kind="ExternalInput")
        for w in ("wq", "wk", "wv")
    }
    wo_img = nc.dram_tensor("wo_img", [128, 2 * D], BF16, kind="ExternalInput")
    tri_in = nc.dram_tensor("tri", [128, 128], F32R, kind="ExternalInput")
    ones_in = nc.dram_tensor("ones4", [128, NHL], F32R, kind="ExternalInput")
    out = nc.dram_tensor("out", [S, D], BF16, kind="ExternalOutput")

    def dram_ap(t, base, ap):
        ref = t[0:1, 0:1]
        return bass.AP(tensor=ref.tensor, offset=base, ap=[list(a) for a in ap])

    with tile.TileContext(nc) as tc:
        with (
            tc.tile_pool(name="persist", bufs=1) as pp,
            tc.tile_pool(name="work", bufs=3) as wp,
            tc.tile_pool(name="ps", bufs=1, space="PSUM") as ps,
        ):
            # ---- mega tiles ----
            xm8 = pp.tile([128, DT * S], E4M3, name="xm8", tag="xm8")
            xm84 = xm8.rearrange("p (j t s) -> p j t s", j=NJ, t=2)
            dxm8 = pp.tile([128, DT * S], E5M2, name="dxm8", tag="dxm8")
            dxm84 = dxm8.rearrange("p (j t s) -> p j t s", j=NJ, t=2)
            w84 = {}
            dw84 = {}
            for w in ("wq", "wk", "wv"):
                t8 = pp.tile([128, DT * E], E4M3, name=f"{w}8", tag=f"{w}8")
                w84[w] = t8.rearrange("p (j t e) -> p j t e", j=NJ, t=2)
                td = pp.tile([128, DT * E], E5M2, name=f"d{w}8", tag=f"d{w}8")
                dw84[w] = td.rearrange("p (j t e) -> p j t e", j=NJ, t=2)
            wom = pp.tile([128, 2 * D], BF16, name="wom", tag="wom")
            wom3 = wom.rearrange("p (d c) -> p d c", d=2)
            tri = pp.tile([128, 128], F32R, name="tri", tag="tri")
            tri_bf = pp.tile([128, 128], BF16, name="tri_bf", tag="tri_bf")
            ones_col = pp.tile([128, NHL], F32R, name="ones_col", tag="ones_col")

            # ---- loads: strided DMAs straight off the host images, in
            # first-use order. x on Pool/SWDGE, weights on SP/HWDGE,
            # constants on the ACT queue.
            def x_dma(img, dst4, jlo, jhi, slo, shi):
                nc.gpsimd.dma_start(
                    out=dst4[:, jlo:jhi, :, slo:shi],
                    in_=dram_ap(
                        img,
                        jlo * 2 * S + slo,
                        [[DT * S, 128], [S, 2 * (jhi - jlo)], [1, shi - slo]],
                    ),
                )

            def w_dma(wdram, dst4, jlo, jhi):
                nc.sync.dma_start(
                    out=dst4[:, jlo:jhi, :, :],
                    in_=dram_ap(
                        wdram,
                        jlo * 2 * E,
                        [[DT * E, 128], [1, 2 * (jhi - jlo) * E]],
                    ),
                )

            w_dma(w8_imgs["wq"], w84["wq"], 0, 2)
            x_dma(x8_img, xm84, 0, 2, 0, QBS)
            w_dma(w8_imgs["wq"], w84["wq"], 2, 4)
            x_dma(x8_img, xm84, 2, 4, 0, QBS)
            w_dma(w8_imgs["wk"], w84["wk"], 0, 4)
            x_dma(dx8_img, dxm84, 0, 2, 0, QBS)
            w_dma(dw8_imgs["wq"], dw84["wq"], 0, 4)
            x_dma(dx8_img, dxm84, 2, 4, 0, QBS)
            w_dma(dw8_imgs["wk"], dw84["wk"], 0, 4)
            w_dma(w8_imgs["wv"], w84["wv"], 0, 4)
            w_dma(dw8_imgs["wv"], dw84["wv"], 0, 4)
            nc.scalar.dma_start(out=tri, in_=tri_in[:, :])
            nc.scalar.dma_start(out=ones_col, in_=ones_in[:, :])
            nc.vector.tensor_copy(out=tri_bf, in_=tri)
            x_dma(x8_img, xm84, 0, 4, QBS, 2 * QBS)
            x_dma(dx8_img, dxm84, 0, 4, QBS, 2 * QBS)
            nc.sync.dma_start(
                out=wom3[:, :, :],
                in_=dram_ap(wo_img, 0, [[2 * D, 128], [1, 2 * D]]),
            )
            x_dma(x8_img, xm84, 0, 4, 2 * QBS, 3 * QBS)
            x_dma(dx8_img, dxm84, 0, 4, 2 * QBS, 3 * QBS)
            x_dma(x8_img, xm84, 0, 4, 3 * QBS, 4 * QBS)
            x_dma(dx8_img, dxm84, 0, 4, 3 * QBS, 4 * QBS)

            # ---- persistent intermediates ----
            q_T = [pp.tile([128, S], BF16, name=f"qT{p}", tag=f"qT{p}") for p in range(2)]
            k_T = [pp.tile([128, S], BF16, name=f"kT{p}", tag=f"kT{p}") for p in range(2)]
            v_aug = [
                pp.tile([128, NHL * (HD + 1)], BF16, name=f"va{st}", tag=f"va{st}")
                for st in range(ST)
            ]
            ctx_T = [pp.tile([128, S], BF16, name=f"cT{p}", tag=f"cT{p}") for p in range(2)]

            # ---- unit builders: each unit is ~2 DoubleRow matmuls or one
            # copy. Projections accumulate 3 compensated fp8 terms:
            # W8^T x8 + W8^T dx8 + dW8^T x8 (12 DR matmuls over 4 kt-pairs),
            # ordered mains-first so the residual images can arrive later.
            def qk_units(p, nb, wkey, dst):
                sl = slice(nb * QBS, (nb + 1) * QBS)
                w4, dw4 = w84[wkey], dw84[wkey]
                psl = slice(p * 128, (p + 1) * 128)
                cell = {}

                def terms(j):
                    return (
                        (w4[:, j, :, psl], xm84[:, j, :, sl]),
                        (w4[:, j, :, psl], dxm84[:, j, :, sl]),
                        (dw4[:, j, :, psl], xm84[:, j, :, sl]),
                    )

                # (term, j) emission order: mains j0..3, then residuals
                order = [(0, j) for j in range(NJ)]
                order += [(t, j) for j in range(NJ) for t in (1, 2)]

                def mk(lo, hi):
                    def u():
                        if lo == 0:
                            cell["acc"] = ps.tile(
                                [128, QBS], F32, name="acc", tag="qk", bufs=2
                            )
                        for i in range(lo, hi):
                            t, j = order[i]
                            lh, rh = terms(j)[t]
                            nc.tensor.matmul(
                                cell["acc"],
                                lhsT=lh,
                                rhs=rh,
                                start=(i == 0),
                                stop=(i == len(order) - 1),
                                perf_mode=DR,
                            )

                    return u

                units = [mk(0, 2), mk(2, 4), mk(4, 6), mk(6, 8), mk(8, 10), mk(10, 12)]

                def fin():
                    nc.vector.tensor_copy(out=dst[p][:, sl], in_=cell["acc"])

                units.append(fin)
                return units

            def v_units(st):
                stsl = slice(st * 128, (st + 1) * 128)
                cell = {}

                def terms(j):
                    return (
                        (xm84[:, j, :, stsl], w84["wv"][:, j, :, :]),
                        (dxm84[:, j, :, stsl], w84["wv"][:, j, :, :]),
                        (xm84[:, j, :, stsl], dw84["wv"][:, j, :, :]),
                    )

                order = [(0, j) for j in range(NJ)]
                order += [(t, j) for j in range(NJ) for t in (1, 2)]

                def mk(lo, hi):
                    def u():
                        if lo == 0:
                            cell["acc"] = ps.tile(
                                [128, QBS], F32, name="acc", tag="qk", bufs=2
                            )
                        for i in range(lo, hi):
                            t, j = order[i]
                            lh, rh = terms(j)[t]
                            nc.tensor.matmul(
                                cell["acc"][:, 0:E],
                                lhsT=lh,
                                rhs=rh,
                                start=(i == 0),
                                stop=(i == len(order) - 1),
                                perf_mode=DR,
                            )

                    return u

                units = [mk(0, 4), mk(4, 8), mk(8, 12)]

                def fin():
                    va = v_aug[st].rearrange("p (h c) -> p h c", h=NHL)
                    nc.vector.tensor_copy(
                        out=va[:, :, 0:HD],
                        in_=cell["acc"][:, 0:E].rearrange("p (h c) -> p h c", h=NHL),
                    )
                    nc.vector.tensor_copy(
                        out=va[:, :, HD : HD + 1],
                        in_=ones_col.rearrange("p (h c) -> p h c", c=1),
                    )

                units.append(fin)
                return units

            eng_mode = {"drain": False}

            def outproj_units(st, tag="qk", copy_eng=None, tail=False):
                cell = {}

                def mk_mm(nb):
                    def u():
                        pso = ps.tile([128, QBS], F32, name="pso", tag=tag, bufs=2)
                        cell[nb] = pso
                        for dt_ in range(2):
                            nc.tensor.matmul(
                                pso,
                                lhsT=ctx_T[dt_][:, st * 128 : (st + 1) * 128],
                                rhs=wom3[:, dt_, nb * QBS : (nb + 1) * QBS],
                                start=(dt_ == 0),
                                stop=(dt_ == 1),
                            )

                    return u

                def mk_fin(nb, eng):
                    def u():
                        # stage via SBUF bf16 (frees the PSUM slot fast) and
                        # DMA the half right away so the tail's last DMA
                        # chain is short
                        if "osb" not in cell:
                            cell["osb"] = wp.tile(
                                [128, D], BF16, name="osb", tag="osb", bufs=4
                            )
                        if eng == "scalar":
                            nc.scalar.copy(
                                out=cell["osb"][:, nb * QBS : (nb + 1) * QBS],
                                in_=cell[nb],
                            )
                        else:
                            nc.vector.tensor_copy(
                                out=cell["osb"][:, nb * QBS : (nb + 1) * QBS],
                                in_=cell[nb],
                            )
                        # tail: odd-nb DMAs go out the SWDGE (Pool) queue so
                        # the 625ns/DMA HWDGE generation chain halves
                        dma_q = nc.gpsimd if (tail and nb == 1) else nc.sync
                        dma_q.dma_start(
                            out=out[st * 128 : (st + 1) * 128, nb * QBS : (nb + 1) * QBS],
                            in_=cell["osb"][:, nb * QBS : (nb + 1) * QBS],
                        )

                    return u

                if tail:
                    # both matmuls back-to-back (alternating PSUM tags give 4
                    # slots), staging copies split across ACT and DVE, and a
                    # single full-row DMA per st (fewer DGE generations on
                    # the critical tail)
                    def copy_only(nb, eng):
                        def u():
                            if "osb" not in cell:
                                cell["osb"] = wp.tile(
                                    [128, D], BF16, name="osb", tag="osb", bufs=4
                                )
                            dst = cell["osb"][:, nb * QBS : (nb + 1) * QBS]
                            if eng == "scalar":
                                nc.scalar.copy(out=dst, in_=cell[nb])
                            else:
                                nc.vector.tensor_copy(out=dst, in_=cell[nb])

                        return u

                    def full_dma():
                        def u():
                            (nc.gpsimd if st % 2 == 0 else nc.sync).dma_start(
                                out=out[st * 128 : (st + 1) * 128, :],
                                in_=cell["osb"],
                            )

                        return u

                    return [
                        mk_mm(0),
                        mk_mm(1),
                        copy_only(0, "scalar"),
                        copy_only(1, "vector"),
                        full_dma(),
                    ]
                return [mk_mm(0), mk_fin(0, copy_eng), mk_mm(1), mk_fin(1, copy_eng)]

            # ---- attention block with deferred normalization ----
            def attention(p, qb, fillers=(), last=False):
                fillers = list(fillers)
                n_kt = 4 * qb + 4
                ctxs = [
                    ps.tile([128, QBS], F32, name=f"ctx{h}", tag="ctx", bufs=2)
                    for h in range(2)
                ]
                pts = {}
                for kt in range(n_kt + 1):
                    if kt < n_kt:
                        o = 0 if kt < 4 * qb else (kt - 4 * qb) * 128
                        s_ps = ps.tile([128, 2 * QBS], F32, name="s_ps", tag="s", bufs=2)
                        for hl in range(2):
                            nc.tensor.matmul(
                                s_ps[:, hl * QBS + o : (hl + 1) * QBS],
                                lhsT=k_T[p][
                                    hl * HD : (hl + 1) * HD, kt * 128 : (kt + 1) * 128
                                ],
                                rhs=q_T[p][
                                    hl * HD : (hl + 1) * HD,
                                    qb * QBS + o : (qb + 1) * QBS,
                                ],
                                start=True,
                                stop=True,
                            )
                        pt = wp.tile([128, 2 * QBS], BF16, name="pt", tag="pt", bufs=4)
                        sv = s_ps.rearrange("p (h q) -> p h q", h=2)
                        pv = pt.rearrange("p (h q) -> p h q", h=2)
                        nc.scalar.activation(
                            out=pv[:, :, o:QBS],
                            in_=sv[:, :, o:QBS],
                            func=mybir.ActivationFunctionType.Exp,
                            scale=float(SCALE),
                        )
                        if kt >= 4 * qb:
                            for hl in range(2):
                                blk = pt[:, hl * QBS + o : hl * QBS + o + 128]
                                nc.vector.tensor_mul(blk, blk, tri_bf)
                        pts[kt] = (pt, o)
                    # fillers BEFORE ctx(kt-1): PE executes in order, so the
                    # (independent) fillers run while exp(kt-1) finishes; the
                    # ctx matmul then starts without exposing the ACT latency.
                    # Pops adapt so the filler list drains evenly across the
                    # block instead of leaving a burst stuck behind the last
                    # (dependency-carrying) ctx matmul.
                    iters_left = n_kt + 1 - kt
                    if last:
                        npop = 1
                    else:
                        npop = max(2, -(-len(fillers) // iters_left))
                    for _ in range(npop):
                        if fillers:
                            fillers.pop(0)()
                    if kt > 0:
                        pt, o = pts.pop(kt - 1)
                        for hl in range(2):
                            nc.tensor.matmul(
                                ctxs[hl][0 : HD + 1, o:QBS],
                                lhsT=v_aug[kt - 1][
                                    :, (2 * p + hl) * (HD + 1) : (2 * p + hl + 1) * (HD + 1)
                                ],
                                rhs=pt[:, hl * QBS + o : (hl + 1) * QBS],
                                start=(kt - 1 == 0),
                                stop=(kt - 1 == n_kt - 1),
                                skip_group_check=True,
                            )
                # stage unnormalized ctx through SBUF + reciprocal on the
                # denominator row; the broadcast+multiply is deferred
                cuns = []
                for hl in range(2):
                    cun = wp.tile([HD + 1, QBS], F32R, name="cun", tag="cun", bufs=4)
                    if last:
                        # keep the tail's DVE budget for recips/norm muls and
                        # staging copies; ACT has no exps left here
                        nc.scalar.copy(out=cun, in_=ctxs[hl][0 : HD + 1, :])
                    else:
                        nc.vector.tensor_copy(out=cun, in_=ctxs[hl][0 : HD + 1, :])
                    # in-place reciprocal at partition 64 (equal in/out base —
                    # a DVE input at partition 64 with output at partition 0
                    # reads wrong data on HW)
                    with nc.allow_low_precision(reason="f32r is bitwise f32"):
                        nc.vector.reciprocal(
                            out=cun[HD : HD + 1, :], in_=cun[HD : HD + 1, :]
                        )
                    cuns.append(cun)
                if last:
                    # post-loop drain copies go to ACT so DVE stays clear
                    # for the norm chain
                    eng_mode["drain"] = True
                while fillers:
                    fillers.pop(0)()

                bcs = {}

                def mk_bc(hl):
                    cun = cuns[hl]

                    def u():
                        # broadcast recip row across 64 partitions with a
                        # 1-row matmul: ones(1,64)^T @ recip(1,QBS)
                        bc = ps.tile(
                            [128, QBS], F32, name="bc",
                            tag=("s" if last else "qk"), bufs=2,
                        )
                        bcs[hl] = bc
                        # tri row 64 cols 64:128 is all-ones at partition 64,
                        # matching the recip row's base partition
                        nc.tensor.matmul(
                            bc[0:HD, :],
                            lhsT=tri[HD : HD + 1, HD : 2 * HD],
                            rhs=cun[HD : HD + 1, :],
                            start=True,
                            stop=True,
                        )

                    return u

                def mk_mul(hl, sl_):
                    cun = cuns[hl]

                    def u():
                        nc.vector.tensor_mul(
                            ctx_T[p][
                                hl * HD : (hl + 1) * HD,
                                qb * QBS + sl_.start : qb * QBS + sl_.stop,
                            ],
                            cun[0:HD, sl_],
                            bcs[hl][0:HD, sl_],
                        )

                    return u

                def mk_norm(hl):
                    bcu = mk_bc(hl)
                    mulu = mk_mul(hl, slice(0, QBS))

                    def u():
                        bcu()
                        mulu()

                    return u

                if last:
                    # column-split muls: the tail's st12/13 need only the
                    # first half of the qb3 columns
                    return [
                        mk_bc(0),
                        mk_bc(1),
                        mk_mul(0, slice(0, 256)),
                        mk_mul(1, slice(0, 256)),
                        mk_mul(0, slice(256, QBS)),
                        mk_mul(1, slice(256, QBS)),
                    ]
                return [mk_norm(0), mk_norm(1)]

            def with_norm(units, norm):
                units = list(units)
                return units[:4] + list(norm) + units[4:]

            # ---- emission schedule ----
            # startup: mains (w8+x8 only) before residuals so PE consumes in
            # DMA-arrival order; two accs alive at a time (qk tag bufs=2)
            qg = qk_units(0, 0, "wq", q_T)
            kg = qk_units(0, 0, "wk", k_T)
            for u in (qg[0], qg[1], kg[0], kg[1]):
                u()
            for u in qg[2:]:
                u()
            for u in kg[2:]:
                u()
            vg = [v_units(st) for st in range(4)]
            vg[0][0]()
            vg[1][0]()
            for u in vg[0][1:]:
                u()
            for u in vg[1][1:]:
                u()
            vg[2][0]()
            vg[3][0]()
            for u in vg[2][1:]:
                u()
            for u in vg[3][1:]:
                u()
            # att(0,0) gets fillers so its ACT-serial warmup doesn't stall PE;
            # qk(0,1) drains late enough that x block 1 has landed
            n00 = attention(
                0, 0,
                qk_units(1, 0, "wq", q_T) + qk_units(1, 0, "wk", k_T)
                + qk_units(0, 1, "wq", q_T),
            )
            # v(4..7) must be scheduled a block BEFORE att(0,1) reads them
            n10 = attention(
                1, 0,
                with_norm(
                    qk_units(0, 1, "wk", k_T)
                    + v_units(4) + v_units(5) + v_units(6) + v_units(7),
                    n00,
                ),
            )
            n01 = attention(
                0, 1,
                with_norm(
                    qk_units(1, 1, "wq", q_T) + qk_units(1, 1, "wk", k_T), n10
                ),
            )
            n11 = attention(
                1, 1,
                with_norm(
                    qk_units(0, 2, "wq", q_T) + qk_units(0, 2, "wk", k_T)
                    + v_units(8) + v_units(9) + v_units(10) + v_units(11),
                    n01,
                ),
            )
            n02 = attention(
                0, 2,
                with_norm(
                    qk_units(1, 2, "wq", q_T) + qk_units(1, 2, "wk", k_T), n11
                ),
            )
            n12 = attention(
                1, 2,
                with_norm(
                    qk_units(0, 3, "wq", q_T) + qk_units(0, 3, "wk", k_T)
                    + v_units(12) + v_units(13) + v_units(14) + v_units(15)
                    + outproj_units(0) + outproj_units(1),
                    n02,
                ),
            )
            n03 = attention(
                0, 3,
                with_norm(
                    qk_units(1, 3, "wq", q_T) + qk_units(1, 3, "wk", k_T)
                    + outproj_units(2) + outproj_units(3)
                    + outproj_units(4) + outproj_units(5),
                    n12,
                ),
            )
            n13 = attention(
                1, 3,
                with_norm(
                    outproj_units(6) + outproj_units(7) + outproj_units(8)
                    + outproj_units(9) + outproj_units(10) + outproj_units(11),
                    n03,
                ),
                last=True,
            )

            for u in n13[0:4]:
                u()
            # tail: alternate PSUM tags (ctx tag is free now) for a 4-slot
            # rotation; copies split across ACT/DVE, DMAs across HWDGE/SWDGE;
            # st12/13 go right after the first-half norm muls
            for st in (12, 13):
                for u in outproj_units(
                    st, tag=("qk" if st % 2 == 0 else "ctx"), tail=True
                ):
                    u()
            n13[4]()
            n13[5]()
            for st in (14, 15):
                for u in outproj_units(
                    st, tag=("qk" if st % 2 == 0 else "ctx"), tail=True
                ):
                    u()
    return nc


_NC_CACHE = {}


def _get_nc() -> bass.Bass:
    if "nc" not in _NC_CACHE:
        _NC_CACHE["nc"] = build_nc()
    return _NC_CACHE["nc"]


def kernel(in_features: np.ndarray, Wqkv: np.ndarray, Wo: np.ndarray) -> np.ndarray:
    BF = ml_dtypes.bfloat16
    E4 = ml_dtypes.float8_e4m3
    E5 = ml_dtypes.float8_e5m2
    NJ = DT // 2
    x32 = np.ascontiguousarray(np.asarray(in_features, dtype=np.float32))
    Wqkv = np.asarray(Wqkv, dtype=np.float32)
    Wo = np.asarray(Wo, dtype=np.float32)

    tri = np.triu(np.ones((128, 128), dtype=np.float32))  # P^T[k,q] valid iff q >= k

    def img_kpm(arr_t, k, f):
        # arr_t: [k*128, f] -> partition-major image [128, k*f]
        return np.ascontiguousarray(
            arr_t.reshape(k, 128, f).transpose(1, 0, 2).reshape(128, k * f).astype(BF)
        )

    def img_pair(a, f):
        # [DT*128, f] -> kt-paired partition-major image [128, NJ*2*f]
        return np.ascontiguousarray(
            a.reshape(NJ, 2, 128, f).transpose(2, 0, 1, 3).reshape(128, DT * f)
        )

    def split8(arr_t, f):
        # fp8 main (e4m3) + residual (e5m2) pair-layout images
        a8 = arr_t.astype(E4)
        d8 = (arr_t - a8.astype(np.float32)).astype(E5)
        return img_pair(a8, f), img_pair(d8, f)

    in_maps = []
    for c in range(N_CORES):
        b, g = divmod(c, NHL)
        sl = slice(g * E, (g + 1) * E)
        x8, dx8 = split8(x32[b].T, S)
        wq8, dwq8 = split8(np.ascontiguousarray(Wqkv[sl, :]).T, E)
        wk8, dwk8 = split8(np.ascontiguousarray(Wqkv[D:][sl, :]).T, E)
        wv8, dwv8 = split8(np.ascontiguousarray(Wqkv[2 * D :][sl, :]).T, E)
        in_maps.append(
            {
                "x8_img": x8,
                "dx8_img": dx8,
                "wq8_img": wq8,
                "dwq8_img": dwq8,
                "wk8_img": wk8,
                "dwk8_img": dwk8,
                "wv8_img": wv8,
                "dwv8_img": dwv8,
                "wo_img": img_kpm(np.ascontiguousarray(Wo[:, sl]).T, 2, D),
                "tri": tri,
                "ones4": np.ones((128, NHL), dtype=np.float32),
            }
        )

    res = run_bass_kernel_spmd(_get_nc(), in_maps, core_ids=list(range(N_CORES)))
    outs = [res.results[c]["out"].astype(np.float32) for c in range(N_CORES)]
    return np.stack(
        [outs[0] + outs[1] + outs[2] + outs[3], outs[4] + outs[5] + outs[6] + outs[7]],
        axis=0,
    )


# revision 11
# speedup vs baseline: 1.0941x; 1.0941x over previous
"""Causal MHSA Trainium2 kernel (8 NeuronCores) — v7 final.

Sharding: core c = 4*b + g handles batch b and head-group g (4 of 16
heads); host sums the 4 head-group partial projections per batch.

v7 (162.7us stated baseline -> 113.4us, rel err 1.41e-2 vs 2e-2 gate):
- QKV projections as fp8 DoubleRow matmuls (2 kt-tiles per instruction,
  0.5 cycles/col) with residual compensation: W^T x ~= W8^T x8 +
  W8^T dx8 + dW8^T x8; mains e4m3, residuals e5m2 (e4m3 residuals flush
  to subnormals and break the correction). Quantization on host.
- P.V context matmuls for every fully-sub-diagonal kt-pair also run as
  fp8 DoubleRow: exp writes P straight to e4m3 pair tiles, V is staged
  once per kt into e4m3 v_aug8 (pair layout [p, t, h, 68] — dual-fp8
  ldweights needs the outer free step 16B-aligned, hence the padded
  head blocks [v64, one, 0, 0, 0]). Diagonal kt blocks stay bf16.
  Error budget measured on HW: 1.41e-2 (numpy-predicted 1.46e-2).
- Everything else streams bf16 via host-prepared partition-major SBUF
  images; output partials bf16 (host upcasts and sums).
- Scheduling: filler units pop BEFORE each dependency-carrying ctx
  matmul (PE executes in order), adaptive pop counts drain fillers
  evenly, v/outproj filler groups rebalanced into the later (ACT-heavy)
  blocks, startup DMAs ordered mains-first, tail staging split across
  ACT/DVE with single full-row DMAs alternating HWDGE/SWDGE queues and
  column-split last-block normalization.
"""

import json

import ml_dtypes
import numpy as np

import concourse.bass as bass
import concourse.mybir as mybir
import concourse.tile as tile
from concourse.bass_utils import run_bass_kernel_spmd

# ---------------------------------------------------------------------------
# Workaround: this container's walrus rejects instructions carrying more
# than one semaphore wait ("Too many sync wait commands", e.g. on the
# TileContext final drain). Split every multi-wait instruction into
# single-wait NoOps on the same engine placed immediately before it.
# ---------------------------------------------------------------------------


def _split_multiwait_bir(bir_bytes: bytes) -> bytes:
    bir = json.loads(bir_bytes)
    ctr = 0
    for fn in bir.get("functions", []):
        for bb in fn.get("blocks", []):
            out = []
            for inst in bb.get("instructions", []):
                si = inst.get("sync_info")
                waits = (si or {}).get("on_wait") or []
                if len(waits) > 1 and "engine" in inst:
                    for w in waits:
                        ctr += 1
                        out.append(
                            {
                                "debug": inst.get("debug", 0),
                                "engine": inst["engine"],
                                "ins": [],
                                "outs": [],
                                "name": f"{inst['name']}-sw{ctr}",
                                "opcode": "NoOp",
                                "sync_info": {"on_update": [], "on_wait": [w]},
                            }
                        )
                    si["on_wait"] = []
                out.append(inst)
            bb["instructions"] = out
    return json.dumps(bir).encode()


class _BassSplitWaits(bass.Bass):
    def to_json_bytes(self) -> bytes:
        return _split_multiwait_bir(super().to_json_bytes())


# ---------------------------------------------------------------------------
B = 2
S = 2048
D = 1024
HD = 64
N_CORES = 8
NHL = 4  # heads per core
E = NHL * HD  # 256
DT = D // 128  # 8
ST = S // 128  # 16
QBS = 512
NQB = S // QBS  # 4
F32 = mybir.dt.float32
F32R = mybir.dt.float32r
BF16 = mybir.dt.bfloat16
E4M3 = mybir.dt.float8e4
E5M2 = mybir.dt.float8e5
DR = mybir.MatmulPerfMode.DoubleRow
SCALE = 1.0 / np.sqrt(HD)


def build_nc() -> bass.Bass:
    nc = _BassSplitWaits()

    # host-prepared SBUF images (partition-major). QKV runs as fp8
    # DoubleRow with residual compensation: W^T x ~= W8^T x8 + W8^T dx8 +
    # dW8^T x8, where *8 are e4m3 and d* are e5m2 residuals (r = full - *8).
    # Layouts pair kt tiles for DoubleRow: x images are [p, j, t, s]
    # (kt = 2j + t), w images [p, j, t, e].
    NJ = DT // 2  # 4 kt-pairs
    x8_img = nc.dram_tensor("x8_img", [128, DT * S], E4M3, kind="ExternalInput")
    dx8_img = nc.dram_tensor("dx8_img", [128, DT * S], E5M2, kind="ExternalInput")
    w8_imgs = {
        w: nc.dram_tensor(f"{w}8_img", [128, DT * E], E4M3, kind="ExternalInput")
        for w in ("wq", "wk", "wv")
    }
    dw8_imgs = {
        w: nc.dram_tensor(f"d{w}8_img", [128, DT * E], E5M2, kind="ExternalInput")
        for w in ("wq", "wk", "wv")
    }
    wo_img = nc.dram_tensor("wo_img", [128, 2 * D], BF16, kind="ExternalInput")
    tri_in = nc.dram_tensor("tri", [128, 128], F32R, kind="ExternalInput")
    oz_in = nc.dram_tensor("oz8", [128, NHL * 4], F32R, kind="ExternalInput")
    out = nc.dram_tensor("out", [S, D], BF16, kind="ExternalOutput")

    def dram_ap(t, base, ap):
        ref = t[0:1, 0:1]
        return bass.AP(tensor=ref.tensor, offset=base, ap=[list(a) for a in ap])

    with tile.TileContext(nc) as tc:
        with (
            tc.tile_pool(name="persist", bufs=1) as pp,
            tc.tile_pool(name="work", bufs=3) as wp,
            tc.tile_pool(name="ps", bufs=1, space="PSUM") as ps,
        ):
            # ---- mega tiles ----
            xm8 = pp.tile([128, DT * S], E4M3, name="xm8", tag="xm8")
            xm84 = xm8.rearrange("p (j t s) -> p j t s", j=NJ, t=2)
            dxm8 = pp.tile([128, DT * S], E5M2, name="dxm8", tag="dxm8")
            dxm84 = dxm8.rearrange("p (j t s) -> p j t s", j=NJ, t=2)
            w84 = {}
            dw84 = {}
            for w in ("wq", "wk", "wv"):
                t8 = pp.tile([128, DT * E], E4M3, name=f"{w}8", tag=f"{w}8")
                w84[w] = t8.rearrange("p (j t e) -> p j t e", j=NJ, t=2)
                td = pp.tile([128, DT * E], E5M2, name=f"d{w}8", tag=f"d{w}8")
                dw84[w] = td.rearrange("p (j t e) -> p j t e", j=NJ, t=2)
            wom = pp.tile([128, 2 * D], BF16, name="wom", tag="wom")
            wom3 = wom.rearrange("p (d c) -> p d c", d=2)
            tri = pp.tile([128, 128], F32R, name="tri", tag="tri")
            tri_bf = pp.tile([128, 128], BF16, name="tri_bf", tag="tri_bf")
            oz_col = pp.tile([128, NHL * 4], F32R, name="oz_col", tag="oz_col")
            oz4 = oz_col.rearrange("p (h c) -> p h c", c=4)

            # ---- loads: strided DMAs straight off the host images, in
            # first-use order. x on Pool/SWDGE, weights on SP/HWDGE,
            # constants on the ACT queue.
            def x_dma(img, dst4, jlo, jhi, slo, shi):
                nc.gpsimd.dma_start(
                    out=dst4[:, jlo:jhi, :, slo:shi],
                    in_=dram_ap(
                        img,
                        jlo * 2 * S + slo,
                        [[DT * S, 128], [S, 2 * (jhi - jlo)], [1, shi - slo]],
                    ),
                )

            def w_dma(wdram, dst4, jlo, jhi):
                nc.sync.dma_start(
                    out=dst4[:, jlo:jhi, :, :],
                    in_=dram_ap(
                        wdram,
                        jlo * 2 * E,
                        [[DT * E, 128], [1, 2 * (jhi - jlo) * E]],
                    ),
                )

            w_dma(w8_imgs["wq"], w84["wq"], 0, 2)
            x_dma(x8_img, xm84, 0, 2, 0, QBS)
            w_dma(w8_imgs["wq"], w84["wq"], 2, 4)
            x_dma(x8_img, xm84, 2, 4, 0, QBS)
            w_dma(w8_imgs["wk"], w84["wk"], 0, 4)
            x_dma(dx8_img, dxm84, 0, 2, 0, QBS)
            w_dma(dw8_imgs["wq"], dw84["wq"], 0, 4)
            x_dma(dx8_img, dxm84, 2, 4, 0, QBS)
            w_dma(dw8_imgs["wk"], dw84["wk"], 0, 4)
            w_dma(w8_imgs["wv"], w84["wv"], 0, 4)
            w_dma(dw8_imgs["wv"], dw84["wv"], 0, 4)
            nc.scalar.dma_start(out=tri, in_=tri_in[:, :])
            nc.scalar.dma_start(out=oz_col, in_=oz_in[:, :])
            nc.vector.tensor_copy(out=tri_bf, in_=tri)
            x_dma(x8_img, xm84, 0, 4, QBS, 2 * QBS)
            x_dma(dx8_img, dxm84, 0, 4, QBS, 2 * QBS)
            nc.sync.dma_start(
                out=wom3[:, :, :],
                in_=dram_ap(wo_img, 0, [[2 * D, 128], [1, 2 * D]]),
            )
            x_dma(x8_img, xm84, 0, 4, 2 * QBS, 3 * QBS)
            x_dma(dx8_img, dxm84, 0, 4, 2 * QBS, 3 * QBS)
            x_dma(x8_img, xm84, 0, 4, 3 * QBS, 4 * QBS)
            x_dma(dx8_img, dxm84, 0, 4, 3 * QBS, 4 * QBS)

            # ---- persistent intermediates ----
            q_T = [pp.tile([128, S], BF16, name=f"qT{p}", tag=f"qT{p}") for p in range(2)]
            k_T = [pp.tile([128, S], BF16, name=f"kT{p}", tag=f"kT{p}") for p in range(2)]
            v_aug = [
                pp.tile([128, NHL * (HD + 1)], BF16, name=f"va{st}", tag=f"va{st}")
                for st in range(ST)
            ]
            # fp8 copies of V for the kt-pairs whose P.V runs as DoubleRow
            # (every other sub-diagonal pair: kts {0,1},{4,5},{8,9}).
            # Layout [p, t(2), h(4), c(65)] so lhsT slices are [128, 2, 65].
            FP8_PAIRS = (0, 1, 2, 3, 4, 5)  # pair j covers kts 2j, 2j+1
            # dual-fp8 ldweights requires the outer (kt) free step to be
            # even and 16B aligned: layout [p, t, h, c] with c = HD+4 gives
            # a t-stride of 4*68 = 272 bytes. Head block = [v(64), one, 0,0,0].
            v_aug8 = {
                j: pp.tile(
                    [128, 2 * NHL * (HD + 4)], E4M3, name=f"va8_{j}", tag=f"va8_{j}"
                )
                for j in FP8_PAIRS
            }
            ctx_T = [pp.tile([128, S], BF16, name=f"cT{p}", tag=f"cT{p}") for p in range(2)]

            # ---- unit builders: each unit is ~2 DoubleRow matmuls or one
            # copy. Projections accumulate 3 compensated fp8 terms:
            # W8^T x8 + W8^T dx8 + dW8^T x8 (12 DR matmuls over 4 kt-pairs),
            # ordered mains-first so the residual images can arrive later.
            def qk_units(p, nb, wkey, dst):
                sl = slice(nb * QBS, (nb + 1) * QBS)
                w4, dw4 = w84[wkey], dw84[wkey]
                psl = slice(p * 128, (p + 1) * 128)
                cell = {}

                def terms(j):
                    return (
                        (w4[:, j, :, psl], xm84[:, j, :, sl]),
                        (w4[:, j, :, psl], dxm84[:, j, :, sl]),
                        (dw4[:, j, :, psl], xm84[:, j, :, sl]),
                    )

                # (term, j) emission order: mains j0..3, then residuals
                order = [(0, j) for j in range(NJ)]
                order += [(t, j) for j in range(NJ) for t in (1, 2)]

                def mk(lo, hi):
                    def u():
                        if lo == 0:
                            cell["acc"] = ps.tile(
                                [128, QBS], F32, name="acc", tag="qk", bufs=2
                            )
                        for i in range(lo, hi):
                            t, j = order[i]
                            lh, rh = terms(j)[t]
                            nc.tensor.matmul(
                                cell["acc"],
                                lhsT=lh,
                                rhs=rh,
                                start=(i == 0),
                                stop=(i == len(order) - 1),
                                perf_mode=DR,
                            )

                    return u

                units = [mk(0, 2), mk(2, 4), mk(4, 6), mk(6, 8), mk(8, 10), mk(10, 12)]

                def fin():
                    nc.vector.tensor_copy(out=dst[p][:, sl], in_=cell["acc"])

                units.append(fin)
                return units

            def v_units(st):
                stsl = slice(st * 128, (st + 1) * 128)
                cell = {}

                def terms(j):
                    return (
                        (xm84[:, j, :, stsl], w84["wv"][:, j, :, :]),
                        (dxm84[:, j, :, stsl], w84["wv"][:, j, :, :]),
                        (xm84[:, j, :, stsl], dw84["wv"][:, j, :, :]),
                    )

                order = [(0, j) for j in range(NJ)]
                order += [(t, j) for j in range(NJ) for t in (1, 2)]

                def mk(lo, hi):
                    def u():
                        if lo == 0:
                            cell["acc"] = ps.tile(
                                [128, QBS], F32, name="acc", tag="qk", bufs=2
                            )
                        for i in range(lo, hi):
                            t, j = order[i]
                            lh, rh = terms(j)[t]
                            nc.tensor.matmul(
                                cell["acc"][:, 0:E],
                                lhsT=lh,
                                rhs=rh,
                                start=(i == 0),
                                stop=(i == len(order) - 1),
                                perf_mode=DR,
                            )

                    return u

                units = [mk(0, 4), mk(4, 8), mk(8, 12)]

                def fin():
                    va = v_aug[st].rearrange("p (h c) -> p h c", h=NHL)
                    nc.vector.tensor_copy(
                        out=va[:, :, 0:HD],
                        in_=cell["acc"][:, 0:E].rearrange("p (h c) -> p h c", h=NHL),
                    )
                    nc.vector.tensor_copy(
                        out=va[:, :, HD : HD + 1],
                        in_=oz4[:, :, 0:1],
                    )
                    if st // 2 in v_aug8:
                        va8 = v_aug8[st // 2].rearrange(
                            "p (t h c) -> p t h c", t=2, h=NHL
                        )
                        nc.vector.tensor_copy(
                            out=va8[:, st % 2, :, 0:HD],
                            in_=cell["acc"][:, 0:E].rearrange(
                                "p (h c) -> p h c", h=NHL
                            ),
                        )
                        nc.vector.tensor_copy(
                            out=va8[:, st % 2, :, HD : HD + 4],
                            in_=oz4,
                        )

                units.append(fin)
                return units

            eng_mode = {"drain": False}

            def outproj_units(st, tag="qk", copy_eng=None, tail=False):
                cell = {}

                def mk_mm(nb):
                    def u():
                        pso = ps.tile([128, QBS], F32, name="pso", tag=tag, bufs=2)
                        cell[nb] = pso
                        for dt_ in range(2):
                            nc.tensor.matmul(
                                pso,
                                lhsT=ctx_T[dt_][:, st * 128 : (st + 1) * 128],
                                rhs=wom3[:, dt_, nb * QBS : (nb + 1) * QBS],
                                start=(dt_ == 0),
                                stop=(dt_ == 1),
                            )

                    return u

                def mk_fin(nb, eng):
                    def u():
                        # stage via SBUF bf16 (frees the PSUM slot fast) and
                        # DMA the half right away so the tail's last DMA
                        # chain is short
                        if "osb" not in cell:
                            cell["osb"] = wp.tile(
                                [128, D], BF16, name="osb", tag="osb", bufs=4
                            )
                        if eng == "scalar":
                            nc.scalar.copy(
                                out=cell["osb"][:, nb * QBS : (nb + 1) * QBS],
                                in_=cell[nb],
                            )
                        else:
                            nc.vector.tensor_copy(
                                out=cell["osb"][:, nb * QBS : (nb + 1) * QBS],
                                in_=cell[nb],
                            )
                        # tail: odd-nb DMAs go out the SWDGE (Pool) queue so
                        # the 625ns/DMA HWDGE generation chain halves
                        dma_q = nc.gpsimd if (tail and nb == 1) else nc.sync
                        dma_q.dma_start(
                            out=out[st * 128 : (st + 1) * 128, nb * QBS : (nb + 1) * QBS],
                            in_=cell["osb"][:, nb * QBS : (nb + 1) * QBS],
                        )

                    return u

                if tail:
                    # both matmuls back-to-back (alternating PSUM tags give 4
                    # slots), staging copies split across ACT and DVE, and a
                    # single full-row DMA per st (fewer DGE generations on
                    # the critical tail)
                    def copy_only(nb, eng):
                        def u():
                            if "osb" not in cell:
                                cell["osb"] = wp.tile(
                                    [128, D], BF16, name="osb", tag="osb", bufs=4
                                )
                            dst = cell["osb"][:, nb * QBS : (nb + 1) * QBS]
                            if eng == "scalar":
                                nc.scalar.copy(out=dst, in_=cell[nb])
                            else:
                                nc.vector.tensor_copy(out=dst, in_=cell[nb])

                        return u

                    def full_dma():
                        def u():
                            (nc.gpsimd if st % 2 == 0 else nc.sync).dma_start(
                                out=out[st * 128 : (st + 1) * 128, :],
                                in_=cell["osb"],
                            )

                        return u

                    return [
                        mk_mm(0),
                        mk_mm(1),
                        copy_only(0, "scalar"),
                        copy_only(1, "vector"),
                        full_dma(),
                    ]
                return [mk_mm(0), mk_fin(0, copy_eng), mk_mm(1), mk_fin(1, copy_eng)]

            # ---- attention block with deferred normalization ----
            def attention(p, qb, fillers=(), last=False):
                fillers = list(fillers)
                n_kt = 4 * qb + 4
                ctxs = [
                    ps.tile([128, QBS], F32, name=f"ctx{h}", tag="ctx", bufs=2)
                    for h in range(2)
                ]
                def is_fp8(kt):
                    # every fully-sub-diagonal kt-pair runs P.V as fp8
                    # DoubleRow (P from exp in e4m3, V from v_aug8)
                    j = kt // 2
                    return j in v_aug8 and 2 * j + 1 < 4 * qb

                pts = {}
                pt8s = {}
                for kt in range(n_kt + 1):
                    if kt < n_kt:
                        o = 0 if kt < 4 * qb else (kt - 4 * qb) * 128
                        s_ps = ps.tile([128, 2 * QBS], F32, name="s_ps", tag="s", bufs=2)
                        for hl in range(2):
                            nc.tensor.matmul(
                                s_ps[:, hl * QBS + o : (hl + 1) * QBS],
                                lhsT=k_T[p][
                                    hl * HD : (hl + 1) * HD, kt * 128 : (kt + 1) * 128
                                ],
                                rhs=q_T[p][
                                    hl * HD : (hl + 1) * HD,
                                    qb * QBS + o : (qb + 1) * QBS,
                                ],
                                start=True,
                                stop=True,
                            )
                        sv = s_ps.rearrange("p (h q) -> p h q", h=2)
                        if is_fp8(kt):
                            if kt % 2 == 0:
                                pt8s[kt // 2] = wp.tile(
                                    [128, 2 * 2 * QBS], E4M3, name="pt8",
                                    tag="pt8", bufs=2,
                                )
                            pt8v = pt8s[kt // 2].rearrange(
                                "p (t h q) -> p t h q", t=2, h=2
                            )
                            nc.scalar.activation(
                                out=pt8v[:, kt % 2, :, :],
                                in_=sv[:, :, :],
                                func=mybir.ActivationFunctionType.Exp,
                                scale=float(SCALE),
                            )
                        else:
                            pt = wp.tile(
                                [128, 2 * QBS], BF16, name="pt", tag="pt", bufs=4
                            )
                            pv = pt.rearrange("p (h q) -> p h q", h=2)
                            nc.scalar.activation(
                                out=pv[:, :, o:QBS],
                                in_=sv[:, :, o:QBS],
                                func=mybir.ActivationFunctionType.Exp,
                                scale=float(SCALE),
                            )
                            if kt >= 4 * qb:
                                for hl in range(2):
                                    blk = pt[:, hl * QBS + o : hl * QBS + o + 128]
                                    nc.vector.tensor_mul(blk, blk, tri_bf)
                            pts[kt] = (pt, o)
                    # fillers BEFORE ctx(kt-1): PE executes in order, so the
                    # (independent) fillers run while exp(kt-1) finishes; the
                    # ctx matmul then starts without exposing the ACT latency.
                    # Pops adapt so the filler list drains evenly across the
                    # block instead of leaving a burst stuck behind the last
                    # (dependency-carrying) ctx matmul.
                    iters_left = n_kt + 1 - kt
                    if last:
                        npop = 1
                    else:
                        npop = max(2, -(-len(fillers) // iters_left))
                    for _ in range(npop):
                        if fillers:
                            fillers.pop(0)()
                    if kt > 0:
                        prev = kt - 1
                        if is_fp8(prev):
                            if prev % 2 == 1:
                                j = prev // 2
                                pt8v = pt8s.pop(j).rearrange(
                                    "p (t h q) -> p t h q", t=2, h=2
                                )
                                va8 = v_aug8[j].rearrange(
                                    "p (t h c) -> p t h c", t=2, h=NHL
                                )
                                for hl in range(2):
                                    nc.tensor.matmul(
                                        ctxs[hl][0 : HD + 4, :],
                                        lhsT=va8[:, :, 2 * p + hl, :],
                                        rhs=pt8v[:, :, hl, :],
                                        start=(j == 0),
                                        stop=False,
                                        perf_mode=DR,
                                        skip_group_check=True,
                                    )
                        else:
                            pt, o = pts.pop(prev)
                            for hl in range(2):
                                nc.tensor.matmul(
                                    ctxs[hl][0 : HD + 1, o:QBS],
                                    lhsT=v_aug[prev][
                                        :, (2 * p + hl) * (HD + 1) : (2 * p + hl + 1) * (HD + 1)
                                    ],
                                    rhs=pt[:, hl * QBS + o : (hl + 1) * QBS],
                                    start=(prev == 0),
                                    stop=(prev == n_kt - 1),
                                    skip_group_check=True,
                                )
                # stage unnormalized ctx through SBUF + reciprocal on the
                # denominator row; the broadcast+multiply is deferred
                cuns = []
                for hl in range(2):
                    cun = wp.tile([HD + 1, QBS], F32R, name="cun", tag="cun", bufs=4)
                    if last:
                        # keep the tail's DVE budget for recips/norm muls and
                        # staging copies; ACT has no exps left here
                        nc.scalar.copy(out=cun, in_=ctxs[hl][0 : HD + 1, :])
                    else:
                        nc.vector.tensor_copy(out=cun, in_=ctxs[hl][0 : HD + 1, :])
                    # in-place reciprocal at partition 64 (equal in/out base —
                    # a DVE input at partition 64 with output at partition 0
                    # reads wrong data on HW)
                    with nc.allow_low_precision(reason="f32r is bitwise f32"):
                        nc.vector.reciprocal(
                            out=cun[HD : HD + 1, :], in_=cun[HD : HD + 1, :]
                        )
                    cuns.append(cun)
                if last:
                    # post-loop drain copies go to ACT so DVE stays clear
                    # for the norm chain
                    eng_mode["drain"] = True
                while fillers:
                    fillers.pop(0)()

                bcs = {}

                def mk_bc(hl):
                    cun = cuns[hl]

                    def u():
                        # broadcast recip row across 64 partitions with a
                        # 1-row matmul: ones(1,64)^T @ recip(1,QBS)
                        bc = ps.tile(
                            [128, QBS], F32, name="bc",
                            tag=("s" if last else "qk"), bufs=2,
                        )
                        bcs[hl] = bc
                        # tri row 64 cols 64:128 is all-ones at partition 64,
                        # matching the recip row's base partition
                        nc.tensor.matmul(
                            bc[0:HD, :],
                            lhsT=tri[HD : HD + 1, HD : 2 * HD],
                            rhs=cun[HD : HD + 1, :],
                            start=True,
                            stop=True,
                        )

                    return u

                def mk_mul(hl, sl_):
                    cun = cuns[hl]

                    def u():
                        nc.vector.tensor_mul(
                            ctx_T[p][
                                hl * HD : (hl + 1) * HD,
                                qb * QBS + sl_.start : qb * QBS + sl_.stop,
                            ],
                            cun[0:HD, sl_],
                            bcs[hl][0:HD, sl_],
                        )

                    return u

                def mk_norm(hl):
                    bcu = mk_bc(hl)
                    mulu = mk_mul(hl, slice(0, QBS))

                    def u():
                        bcu()
                        mulu()

                    return u

                if last:
                    # column-split muls: the tail's st12/13 need only the
                    # first half of the qb3 columns
                    return [
                        mk_bc(0),
                        mk_bc(1),
                        mk_mul(0, slice(0, 256)),
                        mk_mul(1, slice(0, 256)),
                        mk_mul(0, slice(256, QBS)),
                        mk_mul(1, slice(256, QBS)),
                    ]
                return [mk_norm(0), mk_norm(1)]

            def with_norm(units, norm):
                units = list(units)
                return units[:4] + list(norm) + units[4:]

            # ---- emission schedule ----
            # startup: mains (w8+x8 only) before residuals so PE consumes in
            # DMA-arrival order; two accs alive at a time (qk tag bufs=2)
            qg = qk_units(0, 0, "wq", q_T)
            kg = qk_units(0, 0, "wk", k_T)
            for u in (qg[0], qg[1], kg[0], kg[1]):
                u()
            for u in qg[2:]:
                u()
            for u in kg[2:]:
                u()
            vg = [v_units(st) for st in range(4)]
            vg[0][0]()
            vg[1][0]()
            for u in vg[0][1:]:
                u()
            for u in vg[1][1:]:
                u()
            vg[2][0]()
            vg[3][0]()
            for u in vg[2][1:]:
                u()
            for u in vg[3][1:]:
                u()
            # att(0,0) gets fillers so its ACT-serial warmup doesn't stall PE;
            # qk(0,1) drains late enough that x block 1 has landed
            n00 = attention(
                0, 0,
                qk_units(1, 0, "wq", q_T) + qk_units(1, 0, "wk", k_T)
                + qk_units(0, 1, "wq", q_T),
            )
            # v(4..7) must be scheduled a block BEFORE att(0,1) reads them
            n10 = attention(
                1, 0,
                with_norm(
                    qk_units(0, 1, "wk", k_T)
                    + v_units(4) + v_units(5) + v_units(6) + v_units(7),
                    n00,
                ),
            )
            n01 = attention(
                0, 1,
                with_norm(
                    qk_units(1, 1, "wq", q_T) + qk_units(1, 1, "wk", k_T), n10
                ),
            )
            n11 = attention(
                1, 1,
                with_norm(
                    qk_units(0, 2, "wq", q_T) + qk_units(0, 2, "wk", k_T)
                    + v_units(8) + outproj_units(0),
                    n01,
                ),
            )
            n02 = attention(
                0, 2,
                with_norm(
                    qk_units(1, 2, "wq", q_T) + qk_units(1, 2, "wk", k_T)
                    + v_units(9) + v_units(10) + v_units(11)
                    + outproj_units(1),
                    n11,
                ),
            )
            n12 = attention(
                1, 2,
                with_norm(
                    qk_units(0, 3, "wq", q_T) + qk_units(0, 3, "wk", k_T)
                    + v_units(12) + v_units(13) + v_units(14),
                    n02,
                ),
            )
            n03 = attention(
                0, 3,
                with_norm(
                    qk_units(1, 3, "wq", q_T) + qk_units(1, 3, "wk", k_T)
                    + v_units(15)
                    + outproj_units(2) + outproj_units(3)
                    + outproj_units(4) + outproj_units(5),
                    n12,
                ),
            )
            n13 = attention(
                1, 3,
                with_norm(
                    outproj_units(6) + outproj_units(7) + outproj_units(8)
                    + outproj_units(9) + outproj_units(10) + outproj_units(11),
                    n03,
                ),
                last=True,
            )

            for u in n13[0:4]:
                u()
            # tail: alternate PSUM tags (ctx tag is free now) for a 4-slot
            # rotation; copies split across ACT/DVE, DMAs across HWDGE/SWDGE;
            # st12/13 go right after the first-half norm muls
            for st in (12, 13):
                for u in outproj_units(
                    st, tag=("qk" if st % 2 == 0 else "ctx"), tail=True
                ):
                    u()
            n13[4]()
            n13[5]()
            for st in (14, 15):
                for u in outproj_units(
                    st, tag=("qk" if st % 2 == 0 else "ctx"), tail=True
                ):
                    u()
    return nc


_NC_CACHE = {}


def _get_nc() -> bass.Bass:
    if "nc" not in _NC_CACHE:
        _NC_CACHE["nc"] = build_nc()
    return _NC_CACHE["nc"]


def kernel(in_features: np.ndarray, Wqkv: np.ndarray, Wo: np.ndarray) -> np.ndarray:
    BF = ml_dtypes.bfloat16
    E4 = ml_dtypes.float8_e4m3
    E5 = ml_dtypes.float8_e5m2
    NJ = DT // 2
    x32 = np.ascontiguousarray(np.asarray(in_features, dtype=np.float32))
    Wqkv = np.asarray(Wqkv, dtype=np.float32)
    Wo = np.asarray(Wo, dtype=np.float32)

    tri = np.triu(np.ones((128, 128), dtype=np.float32))  # P^T[k,q] valid iff q >= k

    def img_kpm(arr_t, k, f):
        # arr_t: [k*128, f] -> partition-major image [128, k*f]
        return np.ascontiguousarray(
            arr_t.reshape(k, 128, f).transpose(1, 0, 2).reshape(128, k * f).astype(BF)
        )

    def img_pair(a, f):
        # [DT*128, f] -> kt-paired partition-major image [128, NJ*2*f]
        return np.ascontiguousarray(
            a.reshape(NJ, 2, 128, f).transpose(2, 0, 1, 3).reshape(128, DT * f)
        )

    def split8(arr_t, f):
        # fp8 main (e4m3) + residual (e5m2) pair-layout images
        a8 = arr_t.astype(E4)
        d8 = (arr_t - a8.astype(np.float32)).astype(E5)
        return img_pair(a8, f), img_pair(d8, f)

    in_maps = []
    for c in range(N_CORES):
        b, g = divmod(c, NHL)
        sl = slice(g * E, (g + 1) * E)
        x8, dx8 = split8(x32[b].T, S)
        wq8, dwq8 = split8(np.ascontiguousarray(Wqkv[sl, :]).T, E)
        wk8, dwk8 = split8(np.ascontiguousarray(Wqkv[D:][sl, :]).T, E)
        wv8, dwv8 = split8(np.ascontiguousarray(Wqkv[2 * D :][sl, :]).T, E)
        in_maps.append(
            {
                "x8_img": x8,
                "dx8_img": dx8,
                "wq8_img": wq8,
                "dwq8_img": dwq8,
                "wk8_img": wk8,
                "dwk8_img": dwk8,
                "wv8_img": wv8,
                "dwv8_img": dwv8,
                "wo_img": img_kpm(np.ascontiguousarray(Wo[:, sl]).T, 2, D),
                "tri": tri,
                "oz8": np.tile(np.array([1.0, 0.0, 0.0, 0.0], dtype=np.float32), (128, NHL)),
            }
        )

    res = run_bass_kernel_spmd(_get_nc(), in_maps, core_ids=list(range(N_CORES)))
    outs = [res.results[c]["out"].astype(np.float32) for c in range(N_CORES)]
    return np.stack(
        [outs[0] + outs[1] + outs[2] + outs[3], outs[4] + outs[5] + outs[6] + outs[7]],
        axis=0,
    )


# revision 12
# speedup vs baseline: 1.0951x; 1.0009x over previous
"""Causal MHSA Trainium2 kernel (8 NeuronCores) — v4.

Sharding: core c = 4*b + g handles batch b and head-group g (4 of 16
heads); host sums the 4 head-group partial projections per batch.

v4 (vs v3):
- All streaming tensors are bf16: x / Wq / Wk / Wv / Wo inputs arrive as
  host-prepared bf16 SBUF images (one strided DMA each, 2-4KB rows), and
  the output partial is written bf16 (host upcasts and sums). Total DMA
  drops from ~20MB to ~8MB per core, shrinking the startup window and the
  tail drain.
- q_T/k_T/ctx_T live in bf16, so the diagonal score matmuls no longer
  need >=256-col widening (bf16 runs 1 cycle/col at any width).
- Output staging is always through SBUF (bf16), never direct from PSUM.
"""

import json

import ml_dtypes
import numpy as np

import concourse.bass as bass
import concourse.mybir as mybir
import concourse.tile as tile
from concourse.bass_utils import run_bass_kernel_spmd

# ---------------------------------------------------------------------------
# Workaround: this container's walrus rejects instructions carrying more
# than one semaphore wait ("Too many sync wait commands", e.g. on the
# TileContext final drain). Split every multi-wait instruction into
# single-wait NoOps on the same engine placed immediately before it.
# ---------------------------------------------------------------------------


def _split_multiwait_bir(bir_bytes: bytes) -> bytes:
    bir = json.loads(bir_bytes)
    ctr = 0
    for fn in bir.get("functions", []):
        for bb in fn.get("blocks", []):
            out = []
            for inst in bb.get("instructions", []):
                si = inst.get("sync_info")
                waits = (si or {}).get("on_wait") or []
                if len(waits) > 1 and "engine" in inst:
                    for w in waits:
                        ctr += 1
                        out.append(
                            {
                                "debug": inst.get("debug", 0),
                                "engine": inst["engine"],
                                "ins": [],
                                "outs": [],
                                "name": f"{inst['name']}-sw{ctr}",
                                "opcode": "NoOp",
                                "sync_info": {"on_update": [], "on_wait": [w]},
                            }
                        )
                    si["on_wait"] = []
                out.append(inst)
            bb["instructions"] = out
    return json.dumps(bir).encode()


class _BassSplitWaits(bass.Bass):
    def to_json_bytes(self) -> bytes:
        return _split_multiwait_bir(super().to_json_bytes())


# ---------------------------------------------------------------------------
B = 2
S = 2048
D = 1024
HD = 64
N_CORES = 8
NHL = 4  # heads per core
E = NHL * HD  # 256
DT = D // 128  # 8
ST = S // 128  # 16
QBS = 512
NQB = S // QBS  # 4
F32 = mybir.dt.float32
F32R = mybir.dt.float32r
BF16 = mybir.dt.bfloat16
E4M3 = mybir.dt.float8e4
E5M2 = mybir.dt.float8e5
DR = mybir.MatmulPerfMode.DoubleRow
SCALE = 1.0 / np.sqrt(HD)


def build_nc() -> bass.Bass:
    nc = _BassSplitWaits()

    # host-prepared SBUF images (partition-major). QKV runs as fp8
    # DoubleRow with residual compensation: W^T x ~= W8^T x8 + W8^T dx8 +
    # dW8^T x8, where *8 are e4m3 and d* are e5m2 residuals (r = full - *8).
    # Layouts pair kt tiles for DoubleRow: x images are [p, j, t, s]
    # (kt = 2j + t), w images [p, j, t, e].
    NJ = DT // 2  # 4 kt-pairs
    x8_img = nc.dram_tensor("x8_img", [128, DT * S], E4M3, kind="ExternalInput")
    dx8_img = nc.dram_tensor("dx8_img", [128, DT * S], E5M2, kind="ExternalInput")
    w8_imgs = {
        w: nc.dram_tensor(f"{w}8_img", [128, DT * E], E4M3, kind="ExternalInput")
        for w in ("wq", "wk", "wv")
    }
    dw8_imgs = {
        w: nc.dram_tensor(f"d{w}8_img", [128, DT * E], E5M2, kind="ExternalInput")
        for w in ("wq", "wk", "wv")
    }
    wo_img = nc.dram_tensor("wo_img", [128, 2 * D], BF16, kind="ExternalInput")
    tri_in = nc.dram_tensor("tri", [128, 128], F32R, kind="ExternalInput")
    oz_in = nc.dram_tensor("oz8", [128, NHL * 4], F32R, kind="ExternalInput")
    out = nc.dram_tensor("out", [S, D], BF16, kind="ExternalOutput")

    def dram_ap(t, base, ap):
        ref = t[0:1, 0:1]
        return bass.AP(tensor=ref.tensor, offset=base, ap=[list(a) for a in ap])

    with tile.TileContext(nc) as tc:
        with (
            tc.tile_pool(name="persist", bufs=1) as pp,
            tc.tile_pool(name="work", bufs=3) as wp,
            tc.tile_pool(name="ps", bufs=1, space="PSUM") as ps,
        ):
            # ---- mega tiles ----
            xm8 = pp.tile([128, DT * S], E4M3, name="xm8", tag="xm8")
            xm84 = xm8.rearrange("p (j t s) -> p j t s", j=NJ, t=2)
            dxm8 = pp.tile([128, DT * S], E5M2, name="dxm8", tag="dxm8")
            dxm84 = dxm8.rearrange("p (j t s) -> p j t s", j=NJ, t=2)
            w84 = {}
            dw84 = {}
            for w in ("wq", "wk", "wv"):
                t8 = pp.tile([128, DT * E], E4M3, name=f"{w}8", tag=f"{w}8")
                w84[w] = t8.rearrange("p (j t e) -> p j t e", j=NJ, t=2)
                td = pp.tile([128, DT * E], E5M2, name=f"d{w}8", tag=f"d{w}8")
                dw84[w] = td.rearrange("p (j t e) -> p j t e", j=NJ, t=2)
            wom = pp.tile([128, 2 * D], BF16, name="wom", tag="wom")
            wom3 = wom.rearrange("p (d c) -> p d c", d=2)
            tri = pp.tile([128, 128], F32R, name="tri", tag="tri")
            tri_bf = pp.tile([128, 128], BF16, name="tri_bf", tag="tri_bf")
            oz_col = pp.tile([128, NHL * 4], F32R, name="oz_col", tag="oz_col")
            oz4 = oz_col.rearrange("p (h c) -> p h c", c=4)

            # ---- loads: strided DMAs straight off the host images, in
            # first-use order. x on Pool/SWDGE, weights on SP/HWDGE,
            # constants on the ACT queue.
            def x_dma(img, dst4, jlo, jhi, slo, shi):
                nc.gpsimd.dma_start(
                    out=dst4[:, jlo:jhi, :, slo:shi],
                    in_=dram_ap(
                        img,
                        jlo * 2 * S + slo,
                        [[DT * S, 128], [S, 2 * (jhi - jlo)], [1, shi - slo]],
                    ),
                )

            def w_dma(wdram, dst4, jlo, jhi):
                nc.sync.dma_start(
                    out=dst4[:, jlo:jhi, :, :],
                    in_=dram_ap(
                        wdram,
                        jlo * 2 * E,
                        [[DT * E, 128], [1, 2 * (jhi - jlo) * E]],
                    ),
                )

            w_dma(w8_imgs["wq"], w84["wq"], 0, 2)
            x_dma(x8_img, xm84, 0, 2, 0, QBS)
            w_dma(w8_imgs["wq"], w84["wq"], 2, 4)
            x_dma(x8_img, xm84, 2, 4, 0, QBS)
            w_dma(w8_imgs["wk"], w84["wk"], 0, 4)
            x_dma(dx8_img, dxm84, 0, 2, 0, QBS)
            w_dma(dw8_imgs["wq"], dw84["wq"], 0, 4)
            x_dma(dx8_img, dxm84, 2, 4, 0, QBS)
            w_dma(dw8_imgs["wk"], dw84["wk"], 0, 4)
            w_dma(w8_imgs["wv"], w84["wv"], 0, 4)
            w_dma(dw8_imgs["wv"], dw84["wv"], 0, 4)
            nc.scalar.dma_start(out=tri, in_=tri_in[:, :])
            nc.scalar.dma_start(out=oz_col, in_=oz_in[:, :])
            nc.vector.tensor_copy(out=tri_bf, in_=tri)
            x_dma(x8_img, xm84, 0, 4, QBS, 2 * QBS)
            x_dma(dx8_img, dxm84, 0, 4, QBS, 2 * QBS)
            nc.sync.dma_start(
                out=wom3[:, :, :],
                in_=dram_ap(wo_img, 0, [[2 * D, 128], [1, 2 * D]]),
            )
            x_dma(x8_img, xm84, 0, 4, 2 * QBS, 3 * QBS)
            x_dma(dx8_img, dxm84, 0, 4, 2 * QBS, 3 * QBS)
            x_dma(x8_img, xm84, 0, 4, 3 * QBS, 4 * QBS)
            x_dma(dx8_img, dxm84, 0, 4, 3 * QBS, 4 * QBS)

            # ---- persistent intermediates ----
            q_T = [pp.tile([128, S], BF16, name=f"qT{p}", tag=f"qT{p}") for p in range(2)]
            k_T = [pp.tile([128, S], BF16, name=f"kT{p}", tag=f"kT{p}") for p in range(2)]
            v_aug = [
                pp.tile([128, NHL * (HD + 1)], BF16, name=f"va{st}", tag=f"va{st}")
                for st in range(ST)
            ]
            # fp8 copies of V for the kt-pairs whose P.V runs as DoubleRow
            # (every other sub-diagonal pair: kts {0,1},{4,5},{8,9}).
            # Layout [p, t(2), h(4), c(65)] so lhsT slices are [128, 2, 65].
            FP8_PAIRS = (0, 1, 2, 3, 4, 5)  # pair j covers kts 2j, 2j+1
            # dual-fp8 ldweights requires the outer (kt) free step to be
            # even and 16B aligned: layout [p, t, h, c] with c = HD+4 gives
            # a t-stride of 4*68 = 272 bytes. Head block = [v(64), one, 0,0,0].
            v_aug8 = {
                j: pp.tile(
                    [128, 2 * NHL * (HD + 4)], E4M3, name=f"va8_{j}", tag=f"va8_{j}"
                )
                for j in FP8_PAIRS
            }
            ctx_T = [pp.tile([128, S], BF16, name=f"cT{p}", tag=f"cT{p}") for p in range(2)]

            # ---- unit builders: each unit is ~2 DoubleRow matmuls or one
            # copy. Projections accumulate 3 compensated fp8 terms:
            # W8^T x8 + W8^T dx8 + dW8^T x8 (12 DR matmuls over 4 kt-pairs),
            # ordered mains-first so the residual images can arrive later.
            def qk_units(p, nb, wkey, dst):
                sl = slice(nb * QBS, (nb + 1) * QBS)
                w4, dw4 = w84[wkey], dw84[wkey]
                psl = slice(p * 128, (p + 1) * 128)
                cell = {}

                def terms(j):
                    return (
                        (w4[:, j, :, psl], xm84[:, j, :, sl]),
                        (w4[:, j, :, psl], dxm84[:, j, :, sl]),
                        (dw4[:, j, :, psl], xm84[:, j, :, sl]),
                    )

                # (term, j) emission order: mains j0..3, then residuals
                # (w-residual first: dwq lands before the dx8 j2/j3 chunks)
                order = [(0, j) for j in range(NJ)]
                order += [(t, j) for j in range(NJ) for t in (2, 1)]

                def mk(lo, hi):
                    def u():
                        if lo == 0:
                            cell["acc"] = ps.tile(
                                [128, QBS], F32, name="acc", tag="qk", bufs=2
                            )
                        for i in range(lo, hi):
                            t, j = order[i]
                            lh, rh = terms(j)[t]
                            nc.tensor.matmul(
                                cell["acc"],
                                lhsT=lh,
                                rhs=rh,
                                start=(i == 0),
                                stop=(i == len(order) - 1),
                                perf_mode=DR,
                            )

                    return u

                units = [mk(0, 2), mk(2, 4), mk(4, 6), mk(6, 8), mk(8, 10), mk(10, 12)]

                def fin():
                    nc.vector.tensor_copy(out=dst[p][:, sl], in_=cell["acc"])

                units.append(fin)
                return units

            def v_units(st):
                stsl = slice(st * 128, (st + 1) * 128)
                cell = {}

                def terms(j):
                    return (
                        (xm84[:, j, :, stsl], w84["wv"][:, j, :, :]),
                        (dxm84[:, j, :, stsl], w84["wv"][:, j, :, :]),
                        (xm84[:, j, :, stsl], dw84["wv"][:, j, :, :]),
                    )

                order = [(0, j) for j in range(NJ)]
                order += [(t, j) for j in range(NJ) for t in (1, 2)]

                def mk(lo, hi):
                    def u():
                        if lo == 0:
                            cell["acc"] = ps.tile(
                                [128, QBS], F32, name="acc", tag="qk", bufs=2
                            )
                        for i in range(lo, hi):
                            t, j = order[i]
                            lh, rh = terms(j)[t]
                            nc.tensor.matmul(
                                cell["acc"][:, 0:E],
                                lhsT=lh,
                                rhs=rh,
                                start=(i == 0),
                                stop=(i == len(order) - 1),
                                perf_mode=DR,
                            )

                    return u

                units = [mk(0, 4), mk(4, 8), mk(8, 12)]

                def fin():
                    va = v_aug[st].rearrange("p (h c) -> p h c", h=NHL)
                    nc.vector.tensor_copy(
                        out=va[:, :, 0:HD],
                        in_=cell["acc"][:, 0:E].rearrange("p (h c) -> p h c", h=NHL),
                    )
                    nc.vector.tensor_copy(
                        out=va[:, :, HD : HD + 1],
                        in_=oz4[:, :, 0:1],
                    )
                    if st // 2 in v_aug8:
                        va8 = v_aug8[st // 2].rearrange(
                            "p (t h c) -> p t h c", t=2, h=NHL
                        )
                        nc.vector.tensor_copy(
                            out=va8[:, st % 2, :, 0:HD],
                            in_=cell["acc"][:, 0:E].rearrange(
                                "p (h c) -> p h c", h=NHL
                            ),
                        )
                        nc.vector.tensor_copy(
                            out=va8[:, st % 2, :, HD : HD + 4],
                            in_=oz4,
                        )

                units.append(fin)
                return units

            eng_mode = {"drain": False}

            def outproj_units(st, tag="qk", copy_eng=None, tail=False):
                cell = {}

                def mk_mm(nb):
                    def u():
                        pso = ps.tile([128, QBS], F32, name="pso", tag=tag, bufs=2)
                        cell[nb] = pso
                        for dt_ in range(2):
                            nc.tensor.matmul(
                                pso,
                                lhsT=ctx_T[dt_][:, st * 128 : (st + 1) * 128],
                                rhs=wom3[:, dt_, nb * QBS : (nb + 1) * QBS],
                                start=(dt_ == 0),
                                stop=(dt_ == 1),
                            )

                    return u

                def mk_fin(nb, eng):
                    def u():
                        # stage via SBUF bf16 (frees the PSUM slot fast) and
                        # DMA the half right away so the tail's last DMA
                        # chain is short
                        if "osb" not in cell:
                            cell["osb"] = wp.tile(
                                [128, D], BF16, name="osb", tag="osb", bufs=4
                            )
                        if eng == "scalar":
                            nc.scalar.copy(
                                out=cell["osb"][:, nb * QBS : (nb + 1) * QBS],
                                in_=cell[nb],
                            )
                        else:
                            nc.vector.tensor_copy(
                                out=cell["osb"][:, nb * QBS : (nb + 1) * QBS],
                                in_=cell[nb],
                            )
                        # tail: odd-nb DMAs go out the SWDGE (Pool) queue so
                        # the 625ns/DMA HWDGE generation chain halves
                        dma_q = nc.gpsimd if (tail and nb == 1) else nc.sync
                        dma_q.dma_start(
                            out=out[st * 128 : (st + 1) * 128, nb * QBS : (nb + 1) * QBS],
                            in_=cell["osb"][:, nb * QBS : (nb + 1) * QBS],
                        )

                    return u

                if tail:
                    # both matmuls back-to-back (alternating PSUM tags give 4
                    # slots), staging copies split across ACT and DVE, and a
                    # single full-row DMA per st (fewer DGE generations on
                    # the critical tail)
                    def copy_only(nb, eng):
                        def u():
                            if "osb" not in cell:
                                cell["osb"] = wp.tile(
                                    [128, D], BF16, name="osb", tag="osb", bufs=4
                                )
                            dst = cell["osb"][:, nb * QBS : (nb + 1) * QBS]
                            if eng == "scalar":
                                nc.scalar.copy(out=dst, in_=cell[nb])
                            else:
                                nc.vector.tensor_copy(out=dst, in_=cell[nb])

                        return u

                    def full_dma():
                        def u():
                            (nc.gpsimd if st % 2 == 0 else nc.sync).dma_start(
                                out=out[st * 128 : (st + 1) * 128, :],
                                in_=cell["osb"],
                            )

                        return u

                    return [
                        mk_mm(0),
                        mk_mm(1),
                        copy_only(0, "scalar"),
                        copy_only(1, "vector"),
                        full_dma(),
                    ]
                return [mk_mm(0), mk_fin(0, copy_eng), mk_mm(1), mk_fin(1, copy_eng)]

            # ---- attention block with deferred normalization ----
            def attention(p, qb, fillers=(), last=False):
                fillers = list(fillers)
                n_kt = 4 * qb + 4
                ctxs = [
                    ps.tile([128, QBS], F32, name=f"ctx{h}", tag="ctx", bufs=2)
                    for h in range(2)
                ]
                def is_fp8(kt):
                    # every fully-sub-diagonal kt-pair runs P.V as fp8
                    # DoubleRow (P from exp in e4m3, V from v_aug8)
                    j = kt // 2
                    return j in v_aug8 and 2 * j + 1 < 4 * qb

                pts = {}
                pt8s = {}
                for kt in range(n_kt + 1):
                    if kt < n_kt:
                        o = 0 if kt < 4 * qb else (kt - 4 * qb) * 128
                        s_ps = ps.tile([128, 2 * QBS], F32, name="s_ps", tag="s", bufs=2)
                        for hl in range(2):
                            nc.tensor.matmul(
                                s_ps[:, hl * QBS + o : (hl + 1) * QBS],
                                lhsT=k_T[p][
                                    hl * HD : (hl + 1) * HD, kt * 128 : (kt + 1) * 128
                                ],
                                rhs=q_T[p][
                                    hl * HD : (hl + 1) * HD,
                                    qb * QBS + o : (qb + 1) * QBS,
                                ],
                                start=True,
                                stop=True,
                            )
                        sv = s_ps.rearrange("p (h q) -> p h q", h=2)
                        if is_fp8(kt):
                            if kt % 2 == 0:
                                pt8s[kt // 2] = wp.tile(
                                    [128, 2 * 2 * QBS], E4M3, name="pt8",
                                    tag="pt8", bufs=2,
                                )
                            pt8v = pt8s[kt // 2].rearrange(
                                "p (t h q) -> p t h q", t=2, h=2
                            )
                            nc.scalar.activation(
                                out=pt8v[:, kt % 2, :, :],
                                in_=sv[:, :, :],
                                func=mybir.ActivationFunctionType.Exp,
                                scale=float(SCALE),
                            )
                        else:
                            pt = wp.tile(
                                [128, 2 * QBS], BF16, name="pt", tag="pt", bufs=4
                            )
                            pv = pt.rearrange("p (h q) -> p h q", h=2)
                            nc.scalar.activation(
                                out=pv[:, :, o:QBS],
                                in_=sv[:, :, o:QBS],
                                func=mybir.ActivationFunctionType.Exp,
                                scale=float(SCALE),
                            )
                            if kt >= 4 * qb:
                                for hl in range(2):
                                    blk = pt[:, hl * QBS + o : hl * QBS + o + 128]
                                    nc.vector.tensor_mul(blk, blk, tri_bf)
                            pts[kt] = (pt, o)
                    # fillers BEFORE ctx(kt-1): PE executes in order, so the
                    # (independent) fillers run while exp(kt-1) finishes; the
                    # ctx matmul then starts without exposing the ACT latency.
                    # Pops adapt so the filler list drains evenly across the
                    # block instead of leaving a burst stuck behind the last
                    # (dependency-carrying) ctx matmul.
                    iters_left = n_kt + 1 - kt
                    if last:
                        npop = 1
                    else:
                        npop = max(2, -(-len(fillers) // iters_left))
                    for _ in range(npop):
                        if fillers:
                            fillers.pop(0)()
                    if kt > 0:
                        prev = kt - 1
                        if is_fp8(prev):
                            if prev % 2 == 1:
                                j = prev // 2
                                pt8v = pt8s.pop(j).rearrange(
                                    "p (t h q) -> p t h q", t=2, h=2
                                )
                                va8 = v_aug8[j].rearrange(
                                    "p (t h c) -> p t h c", t=2, h=NHL
                                )
                                for hl in range(2):
                                    nc.tensor.matmul(
                                        ctxs[hl][0 : HD + 4, :],
                                        lhsT=va8[:, :, 2 * p + hl, :],
                                        rhs=pt8v[:, :, hl, :],
                                        start=(j == 0),
                                        stop=False,
                                        perf_mode=DR,
                                        skip_group_check=True,
                                    )
                        else:
                            pt, o = pts.pop(prev)
                            for hl in range(2):
                                nc.tensor.matmul(
                                    ctxs[hl][0 : HD + 1, o:QBS],
                                    lhsT=v_aug[prev][
                                        :, (2 * p + hl) * (HD + 1) : (2 * p + hl + 1) * (HD + 1)
                                    ],
                                    rhs=pt[:, hl * QBS + o : (hl + 1) * QBS],
                                    start=(prev == 0),
                                    stop=(prev == n_kt - 1),
                                    skip_group_check=True,
                                )
                # stage unnormalized ctx through SBUF + reciprocal on the
                # denominator row; the broadcast+multiply is deferred
                cuns = []
                for hl in range(2):
                    cun = wp.tile([HD + 1, QBS], F32R, name="cun", tag="cun", bufs=4)
                    if last:
                        # keep the tail's DVE budget for recips/norm muls and
                        # staging copies; ACT has no exps left here
                        nc.scalar.copy(out=cun, in_=ctxs[hl][0 : HD + 1, :])
                    else:
                        nc.vector.tensor_copy(out=cun, in_=ctxs[hl][0 : HD + 1, :])
                    # in-place reciprocal at partition 64 (equal in/out base —
                    # a DVE input at partition 64 with output at partition 0
                    # reads wrong data on HW)
                    with nc.allow_low_precision(reason="f32r is bitwise f32"):
                        nc.vector.reciprocal(
                            out=cun[HD : HD + 1, :], in_=cun[HD : HD + 1, :]
                        )
                    cuns.append(cun)
                if last:
                    # post-loop drain copies go to ACT so DVE stays clear
                    # for the norm chain
                    eng_mode["drain"] = True
                while fillers:
                    fillers.pop(0)()

                bcs = {}

                def mk_bc(hl):
                    cun = cuns[hl]

                    def u():
                        # broadcast recip row across 64 partitions with a
                        # 1-row matmul: ones(1,64)^T @ recip(1,QBS)
                        bc = ps.tile(
                            [128, QBS], F32, name="bc",
                            tag=("s" if last else "qk"), bufs=2,
                        )
                        bcs[hl] = bc
                        # tri row 64 cols 64:128 is all-ones at partition 64,
                        # matching the recip row's base partition
                        nc.tensor.matmul(
                            bc[0:HD, :],
                            lhsT=tri[HD : HD + 1, HD : 2 * HD],
                            rhs=cun[HD : HD + 1, :],
                            start=True,
                            stop=True,
                        )

                    return u

                def mk_mul(hl, sl_):
                    cun = cuns[hl]

                    def u():
                        nc.vector.tensor_mul(
                            ctx_T[p][
                                hl * HD : (hl + 1) * HD,
                                qb * QBS + sl_.start : qb * QBS + sl_.stop,
                            ],
                            cun[0:HD, sl_],
                            bcs[hl][0:HD, sl_],
                        )

                    return u

                def mk_norm(hl):
                    bcu = mk_bc(hl)
                    mulu = mk_mul(hl, slice(0, QBS))

                    def u():
                        bcu()
                        mulu()

                    return u

                if last:
                    # column-split muls: the tail's st12/13 need only the
                    # first half of the qb3 columns
                    return [
                        mk_bc(0),
                        mk_bc(1),
                        mk_mul(0, slice(0, 256)),
                        mk_mul(1, slice(0, 256)),
                        mk_mul(0, slice(256, QBS)),
                        mk_mul(1, slice(256, QBS)),
                    ]
                return [mk_norm(0), mk_norm(1)]

            def with_norm(units, norm):
                units = list(units)
                return units[:4] + list(norm) + units[4:]

            # ---- emission schedule ----
            # startup: mains (w8+x8 only) before residuals so PE consumes in
            # DMA-arrival order; two accs alive at a time (qk tag bufs=2)
            qg = qk_units(0, 0, "wq", q_T)
            kg = qk_units(0, 0, "wk", k_T)
            for u in (qg[0], qg[1], kg[0], kg[1]):
                u()
            for u in qg[2:]:
                u()
            for u in kg[2:]:
                u()
            vg = [v_units(st) for st in range(4)]
            vg[0][0]()
            vg[1][0]()
            for u in vg[0][1:]:
                u()
            for u in vg[1][1:]:
                u()
            vg[2][0]()
            vg[3][0]()
            for u in vg[2][1:]:
                u()
            for u in vg[3][1:]:
                u()
            # att(0,0) gets fillers so its ACT-serial warmup doesn't stall PE;
            # qk(0,1) drains late enough that x block 1 has landed
            n00 = attention(
                0, 0,
                qk_units(1, 0, "wq", q_T) + qk_units(1, 0, "wk", k_T)
                + qk_units(0, 1, "wq", q_T),
            )
            # v(4..7) must be scheduled a block BEFORE att(0,1) reads them
            n10 = attention(
                1, 0,
                with_norm(
                    qk_units(0, 1, "wk", k_T)
                    + v_units(4) + v_units(5) + v_units(6) + v_units(7),
                    n00,
                ),
            )
            n01 = attention(
                0, 1,
                with_norm(
                    qk_units(1, 1, "wq", q_T) + qk_units(1, 1, "wk", k_T), n10
                ),
            )
            n11 = attention(
                1, 1,
                with_norm(
                    qk_units(0, 2, "wq", q_T) + qk_units(0, 2, "wk", k_T)
                    + v_units(8) + outproj_units(0),
                    n01,
                ),
            )
            n02 = attention(
                0, 2,
                with_norm(
                    qk_units(1, 2, "wq", q_T) + qk_units(1, 2, "wk", k_T)
                    + v_units(9) + v_units(10) + v_units(11)
                    + outproj_units(1),
                    n11,
                ),
            )
            n12 = attention(
                1, 2,
                with_norm(
                    qk_units(0, 3, "wq", q_T) + qk_units(0, 3, "wk", k_T)
                    + v_units(12) + v_units(13) + v_units(14),
                    n02,
                ),
            )
            n03 = attention(
                0, 3,
                with_norm(
                    qk_units(1, 3, "wq", q_T) + qk_units(1, 3, "wk", k_T)
                    + v_units(15)
                    + outproj_units(2) + outproj_units(3)
                    + outproj_units(4) + outproj_units(5),
                    n12,
                ),
            )
            n13 = attention(
                1, 3,
                with_norm(
                    outproj_units(6) + outproj_units(7) + outproj_units(8)
                    + outproj_units(9) + outproj_units(10) + outproj_units(11),
                    n03,
                ),
                last=True,
            )

            for u in n13[0:4]:
                u()
            # tail: alternate PSUM tags (ctx tag is free now) for a 4-slot
            # rotation; copies split across ACT/DVE, DMAs across HWDGE/SWDGE;
            # st12/13 go right after the first-half norm muls
            for st in (12, 13):
                for u in outproj_units(
                    st, tag=("qk" if st % 2 == 0 else "ctx"), tail=True
                ):
                    u()
            n13[4]()
            n13[5]()
            for st in (14, 15):
                for u in outproj_units(
                    st, tag=("qk" if st % 2 == 0 else "ctx"), tail=True
                ):
                    u()
    return nc


_NC_CACHE = {}


def _get_nc() -> bass.Bass:
    if "nc" not in _NC_CACHE:
        _NC_CACHE["nc"] = build_nc()
    return _NC_CACHE["nc"]


def kernel(in_features: np.ndarray, Wqkv: np.ndarray, Wo: np.ndarray) -> np.ndarray:
    BF = ml_dtypes.bfloat16
    E4 = ml_dtypes.float8_e4m3
    E5 = ml_dtypes.float8_e5m2
    NJ = DT // 2
    x32 = np.ascontiguousarray(np.asarray(in_features, dtype=np.float32))
    Wqkv = np.asarray(Wqkv, dtype=np.float32)
    Wo = np.asarray(Wo, dtype=np.float32)

    tri = np.triu(np.ones((128, 128), dtype=np.float32))  # P^T[k,q] valid iff q >= k

    def img_kpm(arr_t, k, f):
        # arr_t: [k*128, f] -> partition-major image [128, k*f]
        return np.ascontiguousarray(
            arr_t.reshape(k, 128, f).transpose(1, 0, 2).reshape(128, k * f).astype(BF)
        )

    def img_pair(a, f):
        # [DT*128, f] -> kt-paired partition-major image [128, NJ*2*f]
        return np.ascontiguousarray(
            a.reshape(NJ, 2, 128, f).transpose(2, 0, 1, 3).reshape(128, DT * f)
        )

    def split8(arr_t, f):
        # fp8 main (e4m3) + residual (e5m2) pair-layout images
        a8 = arr_t.astype(E4)
        d8 = (arr_t - a8.astype(np.float32)).astype(E5)
        return img_pair(a8, f), img_pair(d8, f)

    in_maps = []
    for c in range(N_CORES):
        b, g = divmod(c, NHL)
        sl = slice(g * E, (g + 1) * E)
        x8, dx8 = split8(x32[b].T, S)
        wq8, dwq8 = split8(np.ascontiguousarray(Wqkv[sl, :]).T, E)
        wk8, dwk8 = split8(np.ascontiguousarray(Wqkv[D:][sl, :]).T, E)
        wv8, dwv8 = split8(np.ascontiguousarray(Wqkv[2 * D :][sl, :]).T, E)
        in_maps.append(
            {
                "x8_img": x8,
                "dx8_img": dx8,
                "wq8_img": wq8,
                "dwq8_img": dwq8,
                "wk8_img": wk8,
                "dwk8_img": dwk8,
                "wv8_img": wv8,
                "dwv8_img": dwv8,
                "wo_img": img_kpm(np.ascontiguousarray(Wo[:, sl]).T, 2, D),
                "tri": tri,
                "oz8": np.tile(np.array([1.0, 0.0, 0.0, 0.0], dtype=np.float32), (128, NHL)),
            }
        )

    res = run_bass_kernel_spmd(_get_nc(), in_maps, core_ids=list(range(N_CORES)))
    outs = [res.results[c]["out"].astype(np.float32) for c in range(N_CORES)]
    return np.stack(
        [outs[0] + outs[1] + outs[2] + outs[3], outs[4] + outs[5] + outs[6] + outs[7]],
        axis=0,
    )
